# revision 21
# baseline (speedup 1.0000x reference)
"""DLRM pairwise-interaction kernel for Trainium2 (8 NeuronCores).

Computes, for each batch b: Z_b = X_b @ X_b^T (X_b is [64, 256]) and emits the
strict lower triangle row-major -> [B, 2016] fp32.

Strategy (pure data parallel over B, 1024 batches per core):
  - fp32 inputs loaded with large HWDGE DMAs (sync ring), cast to fp16 on
    DVE/ACT (alternating); dot-product accumulation stays fp32 in PSUM so the
    end-to-end relative error is ~3e-4.
  - PE-transposes pairs of batches ([128,128] fp16 tiles, identity matmul) to
    put the contraction dim (d) on partitions; DVE evacuates PSUM->SBUF.
  - Per batch: Z = XT^T @ XT as two K=128 accumulating matmuls; two batches
    share the 128 weight columns (column-tiled via output base partition 0/64),
    eight pairs of Z accumulate into one [128, 512] fp32 PSUM bank; DVE copies
    to SBUF with a cast to fp16: z_sb[128 part=(g,i), cols=(pair,j)].
  - Triangle extraction without small DMAs (the v1 bottleneck: per-row
    SBUF->SBUF DMAs all landed on 2 of 16 SDMA engines, and ~550 dma_start
    triggers cost ~1us each on the issuing engines):
      * 64 more PE transposes per supertile flip Z to batch-on-partitions:
        z_t[128 part=pair, cols=(g*64+i, j)] (fp16, exact).
      * 63 compute-engine copies (ACT/Pool alternating, g folded into a 3-dim
        AP) compact rows j<i into out_sb[128 part=pair, (g, 2016)] f32.
      * One 2 MB store per supertile with 16 KB contiguous runs per partition
        spreads across all 16 SDMA engines.
"""
import sys
import numpy as np

sys.path.insert(0, "/opt/trn_rl_repo")

import concourse.bass as bass
import concourse.mybir as mybir
import concourse.tile as tile
from concourse.vector_clock import ScopedClock

F32 = mybir.dt.float32
F16 = mybir.dt.float16

B, N, D, TRI = 8192, 64, 256, 2016
NCORES = 8
NB = B // NCORES           # batches per core
ST_PAIRS = 128             # pairs per supertile (=256 batches)
CHUNK_PAIRS = 8            # pairs per input DMA (1 MB fp32 read)

# ---------------------------------------------------------------------------
# Workaround for walrus builds that only accept ONE sync-wait per instruction:
# hoist all-but-one wait onto NoOp instructions committed just before, on the
# same engine (same-engine program order preserves semantics).
# ---------------------------------------------------------------------------
_orig_commit = tile.TileContext._commit_instruction


def _split_waits(self, inst):
    si = getattr(inst, "sync_info", None)
    if si is None or not si.on_wait or len(si.on_wait) <= 1:
        return
    if inst.engine == mybir.EngineType.Unassigned:
        return
    waits = list(si.on_wait)
    inst.sync_info = mybir.SyncInfo(on_wait=[waits[-1]], on_update=list(si.on_update))
    for w in waits[:-1]:
        nop = mybir.InstNoOp(name=f"{inst.name}-wsplit-{w.id}", ins=[], outs=[])
        nop.engine = inst.engine
        nop.sync_info = mybir.SyncInfo(on_wait=[w], on_update=[])
        _orig_commit(self, nop, lazy_reg_writes=False)


def _commit_instruction_split(self, inst, lazy_reg_writes=True):
    _split_waits(self, inst)
    return _orig_commit(self, inst, lazy_reg_writes=lazy_reg_writes)


def _drain_and_barrier_split(self, tick_clock, wait_clock):
    drain_inst = self.nc.sync.drain()
    wait_clock.add_sem_waits(
        drain_inst.ins, ScopedClock({None: tick_clock.global_clock})
    )
    si = drain_inst.ins.sync_info
    if si is not None and si.on_wait and len(si.on_wait) > 1:
        waits = list(si.on_wait)
        drain_inst.ins.sync_info = mybir.SyncInfo(
            on_wait=[waits[0]], on_update=list(si.on_update)
        )
        for w in waits[1:]:
            nop = self.nc.sync.nop(nofuse=True)
            nop.ins.sync_info = mybir.SyncInfo(on_wait=[w], on_update=[])

    self.nc.all_engine_barrier()
    assert self.sems is not None
    popped = self.nc._tile_sem_poison_stack.pop()
    assert popped is self._sem_poison
    self.nc.clear_and_free_semaphores(list(self.sems.allocated().values()))
    self.nc.all_engine_barrier()


def _install_tile_workarounds():
    tile.TileContext._commit_instruction = _commit_instruction_split
    tile.TileContext._drain_and_barrier = _drain_and_barrier_split


def build_program(nb=NB, st_pairs=ST_PAIRS, chunk_pairs=CHUNK_PAIRS):
    _install_tile_workarounds()
    npairs = nb // 2
    nst = npairs // st_pairs
    nchunks = st_pairs // chunk_pairs
    assert st_pairs == 128

    nc = bass.Bass("TRN2", target_bir_lowering=False, debug=False,
                   num_devices=NCORES)
    x = nc.dram_tensor("x", [nb, N, D], F32, kind="ExternalInput").ap()
    ident = nc.dram_tensor("ident", [128, 128], F16, kind="ExternalInput").ap()
    y = nc.dram_tensor("y", [nb, TRI], F32, kind="ExternalOutput").ap()
    xflat = x.rearrange("b n d -> (b n) d")

    with tile.TileContext(nc) as tc:
        with (
            tc.tile_pool(name="const", bufs=1) as constp,
            tc.tile_pool(name="xf32", bufs=5) as xf32p,
            tc.tile_pool(name="xin", bufs=6) as xinp,
            tc.tile_pool(name="xt", bufs=3) as xtp_sb,
            tc.tile_pool(name="zsb", bufs=2) as zsbp,
            tc.tile_pool(name="zt", bufs=2) as ztp_sb,
            tc.tile_pool(name="osb", bufs=2) as osbp,
            tc.tile_pool(name="xtps", bufs=2, space="PSUM") as xtps,
            tc.tile_pool(name="zps", bufs=3, space="PSUM") as zps,
            tc.tile_pool(name="ztps", bufs=3, space="PSUM") as ztps,
        ):
            ident_sb = constp.tile([128, 128], F16)
            nc.sync.dma_start(ident_sb[:], ident[:])

            for s in range(nst):
                # ---- load X (fp32, sync HWDGE ring) + cast on DVE/ACT ----
                xbufs = []
                for c in range(nchunks):
                    xf = xf32p.tile([128, chunk_pairs * 256], F32, tag="xf32")
                    row0 = (s * st_pairs + c * chunk_pairs) * 128
                    src = xflat[row0:row0 + chunk_pairs * 128, :].rearrange(
                        "(l p) d -> p l d", p=128)
                    nc.sync.dma_start(
                        xf[:].rearrange("p (l d) -> p l d", d=256), src)
                    xb = xinp.tile([128, chunk_pairs * 256], F16, tag="xin")
                    # all casts on ACT: keeps every engine single-phase so a
                    # supertile boundary never head-of-line blocks the next
                    # supertile's early work.
                    nc.scalar.copy(xb[:], xf[:])
                    xbufs.append(xb)

                # ---- Z = X X^T per pair; z_sb fp16 [(g,i), (pair,j)] -------
                z_sb = zsbp.tile([128, st_pairs * 64], F16, tag="zsb")
                for q8 in range(st_pairs // 8):
                    zp = zps.tile([128, 512], F32, tag="zps")
                    for half in range(2):
                        q4 = q8 * 2 + half
                        xtp = xtps.tile([128, 1024], F16, tag="xtps")
                        for pl in range(4):
                            l = q4 * 4 + pl
                            cidx, lc = divmod(l, chunk_pairs)
                            for c in range(2):
                                nc.tensor.transpose(
                                    xtp[:, pl * 256 + c * 128:pl * 256 + (c + 1) * 128],
                                    xbufs[cidx][:, lc * 256 + c * 128:lc * 256 + (c + 1) * 128],
                                    ident_sb[:])
                        xt = xtp_sb.tile([128, 1024], F16, tag="xt")
                        nc.vector.tensor_copy(xt[:], xtp[:])
                        for pl in range(4):
                            slot = half * 4 + pl
                            q0 = pl * 256
                            q1 = pl * 256 + 128
                            zsl = zp[:, slot * 64:(slot + 1) * 64]
                            nc.tensor.matmul(zsl[0:64, :], xt[:, q0:q0 + 64],
                                             xt[:, q0:q0 + 64],
                                             start=True, stop=False,
                                             skip_group_check=True)
                            nc.tensor.matmul(zsl[64:128, :], xt[:, q0 + 64:q0 + 128],
                                             xt[:, q0 + 64:q0 + 128],
                                             start=True, stop=False,
                                             skip_group_check=True)
                            nc.tensor.matmul(zsl[0:64, :], xt[:, q1:q1 + 64],
                                             xt[:, q1:q1 + 64],
                                             start=False, stop=True,
                                             skip_group_check=True)
                            nc.tensor.matmul(zsl[64:128, :], xt[:, q1 + 64:q1 + 128],
                                             xt[:, q1 + 64:q1 + 128],
                                             start=False, stop=True,
                                             skip_group_check=True)
                    nc.vector.tensor_copy(z_sb[:, q8 * 512:(q8 + 1) * 512], zp[:])

                # ---- PE transpose Z -> batch on partitions -----------------
                # z_sb col = pair*64 + j ; per fixed j take [128=(g,i), 128=pair]
                # (stride 64 elems) -> ztp [128=pair, 128=(g*64+i)].
                # z_t col = (g*64+i)*64 + j.
                zview = z_sb[:].rearrange("p (l j) -> p j l", j=64)
                z_t = ztp_sb.tile([128, st_pairs * 64], F16, tag="zt")
                # dest loop order [p, gi, j8]: inner dim = 8 contiguous fp16
                # (16B runs) so DVE writes bursts, not strided singles.
                ztr = z_t[:].rearrange("p (gi j) -> p gi j", j=64)
                out_sb = osbp.tile([128, 2 * TRI], F32, tag="osb")
                osr = out_sb[:].rearrange("p (g t) -> p g t", g=2)
                ztg = z_t[:].rearrange("p (g c) -> p g c", g=2)
                ydst = y[s * 2 * st_pairs:(s + 1) * 2 * st_pairs, :].rearrange(
                    "(p g) t -> p g t", g=2)
                for jb in range(8):
                    ztp = ztps.tile([128, 1024], F16, tag="ztps")
                    for jj in range(8):
                        nc.tensor.transpose(
                            ztp[:, jj * 128:(jj + 1) * 128],
                            zview[:, jb * 8 + jj, :],
                            ident_sb[:])
                    nc.vector.tensor_copy(
                        ztr[:, :, jb * 8:(jb + 1) * 8],
                        ztp[:].rearrange("p (j gi) -> p gi j", j=8))
                    # triangle rows that only need j <= jb*8+7 (ACT/Pool, no
                    # DMA): overlaps the remaining Z transposes.
                    for i in range(jb * 8 + 1, min(jb * 8 + 9, 64)):
                        off = i * (i - 1) // 2
                        dst = osr[:, :, off:off + i]
                        src = ztg[:, :, i * 64:i * 64 + i]
                        nc.gpsimd.tensor_copy(dst, src)
                    # two partial stores overlap the compaction tail; issued
                    # from GpSimd (SWDGE) right behind the compaction copies
                    # they depend on, so no engine queue ever stalls on them.
                    if jb == 3:
                        nc.gpsimd.dma_start(ydst[:, :, 0:528], osr[:, :, 0:528])
                    elif jb == 7:
                        nc.gpsimd.dma_start(ydst[:, :, 528:TRI], osr[:, :, 528:TRI])
    return nc


_PROGRAM_CACHE = {}


def _get_program():
    if "nc" not in _PROGRAM_CACHE:
        _PROGRAM_CACHE["nc"] = build_program()
    return _PROGRAM_CACHE["nc"]


def kernel(inputs):
    from concourse.bass_utils import run_bass_kernel_spmd

    x = np.asarray(inputs, dtype=np.float32)
    assert x.shape == (B, N, D), x.shape
    nc = _get_program()
    eye = np.eye(128, dtype=np.float16)
    in_maps = [
        {"x": np.ascontiguousarray(x[i * NB:(i + 1) * NB]), "ident": eye}
        for i in range(NCORES)
    ]
    res = run_bass_kernel_spmd(nc, in_maps, list(range(NCORES)))
    out = np.concatenate([res.results[i]["y"] for i in range(NCORES)], axis=0)
    return out.astype(np.float32, copy=False)


# revision 22
# speedup vs baseline: 1.0418x; 1.0418x over previous
"""DLRM pairwise-interaction kernel for Trainium2 (8 NeuronCores).

Computes, for each batch b: Z_b = X_b @ X_b^T (X_b is [64, 256]) and emits the
strict lower triangle row-major -> [B, 2016] fp32.

Strategy (pure data parallel over B, 1024 batches per core):
  - fp32 inputs loaded with large HWDGE DMAs (sync ring), cast to fp16 on
    DVE/ACT (alternating); dot-product accumulation stays fp32 in PSUM so the
    end-to-end relative error is ~3e-4.
  - PE-transposes pairs of batches ([128,128] fp16 tiles, identity matmul) to
    put the contraction dim (d) on partitions; DVE evacuates PSUM->SBUF.
  - Per batch: Z = XT^T @ XT as two K=128 accumulating matmuls; two batches
    share the 128 weight columns (column-tiled via output base partition 0/64),
    eight pairs of Z accumulate into one [128, 512] fp32 PSUM bank; DVE copies
    to SBUF with a cast to fp16: z_sb[128 part=(g,i), cols=(pair,j)].
  - Triangle extraction without small DMAs (the v1 bottleneck: per-row
    SBUF->SBUF DMAs all landed on 2 of 16 SDMA engines, and ~550 dma_start
    triggers cost ~1us each on the issuing engines):
      * 64 more PE transposes per supertile flip Z to batch-on-partitions:
        z_t[128 part=pair, cols=(g*64+i, j)] (fp16, exact).
      * 63 compute-engine copies (ACT/Pool alternating, g folded into a 3-dim
        AP) compact rows j<i into out_sb[128 part=pair, (g, 2016)] f32.
      * One 2 MB store per supertile with 16 KB contiguous runs per partition
        spreads across all 16 SDMA engines.
"""
import sys
import numpy as np

sys.path.insert(0, "/opt/trn_rl_repo")

import concourse.bass as bass
import concourse.mybir as mybir
import concourse.tile as tile
from concourse.vector_clock import ScopedClock

F32 = mybir.dt.float32
F16 = mybir.dt.float16

B, N, D, TRI = 8192, 64, 256, 2016
NCORES = 8
NB = B // NCORES           # batches per core
ST_PAIRS = 128             # pairs per supertile (=256 batches)
CHUNK_PAIRS = 8            # pairs per input DMA (1 MB fp32 read)

# ---------------------------------------------------------------------------
# Workaround for walrus builds that only accept ONE sync-wait per instruction:
# hoist all-but-one wait onto NoOp instructions committed just before, on the
# same engine (same-engine program order preserves semantics).
# ---------------------------------------------------------------------------
_orig_commit = tile.TileContext._commit_instruction


def _split_waits(self, inst):
    si = getattr(inst, "sync_info", None)
    if si is None or not si.on_wait or len(si.on_wait) <= 1:
        return
    if inst.engine == mybir.EngineType.Unassigned:
        return
    waits = list(si.on_wait)
    inst.sync_info = mybir.SyncInfo(on_wait=[waits[-1]], on_update=list(si.on_update))
    for w in waits[:-1]:
        nop = mybir.InstNoOp(name=f"{inst.name}-wsplit-{w.id}", ins=[], outs=[])
        nop.engine = inst.engine
        nop.sync_info = mybir.SyncInfo(on_wait=[w], on_update=[])
        _orig_commit(self, nop, lazy_reg_writes=False)


def _commit_instruction_split(self, inst, lazy_reg_writes=True):
    _split_waits(self, inst)
    return _orig_commit(self, inst, lazy_reg_writes=lazy_reg_writes)


def _drain_and_barrier_split(self, tick_clock, wait_clock):
    drain_inst = self.nc.sync.drain()
    wait_clock.add_sem_waits(
        drain_inst.ins, ScopedClock({None: tick_clock.global_clock})
    )
    si = drain_inst.ins.sync_info
    if si is not None and si.on_wait and len(si.on_wait) > 1:
        waits = list(si.on_wait)
        drain_inst.ins.sync_info = mybir.SyncInfo(
            on_wait=[waits[0]], on_update=list(si.on_update)
        )
        for w in waits[1:]:
            nop = self.nc.sync.nop(nofuse=True)
            nop.ins.sync_info = mybir.SyncInfo(on_wait=[w], on_update=[])

    self.nc.all_engine_barrier()
    assert self.sems is not None
    popped = self.nc._tile_sem_poison_stack.pop()
    assert popped is self._sem_poison
    self.nc.clear_and_free_semaphores(list(self.sems.allocated().values()))
    self.nc.all_engine_barrier()


def _install_tile_workarounds():
    tile.TileContext._commit_instruction = _commit_instruction_split
    tile.TileContext._drain_and_barrier = _drain_and_barrier_split


def build_program(nb=NB, st_pairs=ST_PAIRS, chunk_pairs=CHUNK_PAIRS):
    _install_tile_workarounds()
    npairs = nb // 2
    nst = npairs // st_pairs
    nchunks = st_pairs // chunk_pairs
    assert st_pairs == 128

    nc = bass.Bass("TRN2", target_bir_lowering=False, debug=False,
                   num_devices=NCORES)
    x = nc.dram_tensor("x", [nb, N, D], F32, kind="ExternalInput").ap()
    ident = nc.dram_tensor("ident", [128, 128], F16, kind="ExternalInput").ap()
    y = nc.dram_tensor("y", [nb, TRI], F32, kind="ExternalOutput").ap()
    xflat = x.rearrange("b n d -> (b n) d")

    with tile.TileContext(nc) as tc:
        with (
            tc.tile_pool(name="const", bufs=1) as constp,
            tc.tile_pool(name="xf32", bufs=5) as xf32p,
            tc.tile_pool(name="xin", bufs=6) as xinp,
            tc.tile_pool(name="xt", bufs=3) as xtp_sb,
            tc.tile_pool(name="zsb", bufs=2) as zsbp,
            tc.tile_pool(name="zt", bufs=2) as ztp_sb,
            tc.tile_pool(name="osb", bufs=2) as osbp,
            tc.tile_pool(name="xtps", bufs=2, space="PSUM") as xtps,
            tc.tile_pool(name="zps", bufs=3, space="PSUM") as zps,
            tc.tile_pool(name="ztps", bufs=3, space="PSUM") as ztps,
        ):
            ident_sb = constp.tile([128, 128], F16)
            nc.sync.dma_start(ident_sb[:], ident[:])

            for s in range(nst):
                # ---- load X (fp32, sync HWDGE ring) + cast on DVE/ACT ----
                xbufs = []
                for c in range(nchunks):
                    xf = xf32p.tile([128, chunk_pairs * 256], F32, tag="xf32")
                    row0 = (s * st_pairs + c * chunk_pairs) * 128
                    src = xflat[row0:row0 + chunk_pairs * 128, :].rearrange(
                        "(l p) d -> p l d", p=128)
                    nc.sync.dma_start(
                        xf[:].rearrange("p (l d) -> p l d", d=256), src)
                    xb = xinp.tile([128, chunk_pairs * 256], F16, tag="xin")
                    # all casts on ACT: keeps every engine single-phase so a
                    # supertile boundary never head-of-line blocks the next
                    # supertile's early work.
                    nc.scalar.copy(xb[:], xf[:])
                    xbufs.append(xb)

                # ---- Z = X X^T per pair; z_sb fp16 [(g,i), (pair,j)] -------
                z_sb = zsbp.tile([128, st_pairs * 64], F16, tag="zsb")
                for q8 in range(st_pairs // 8):
                    zp = zps.tile([128, 512], F32, tag="zps")
                    for half in range(2):
                        q4 = q8 * 2 + half
                        xtp = xtps.tile([128, 1024], F16, tag="xtps")
                        for pl in range(4):
                            l = q4 * 4 + pl
                            cidx, lc = divmod(l, chunk_pairs)
                            for c in range(2):
                                nc.tensor.transpose(
                                    xtp[:, pl * 256 + c * 128:pl * 256 + (c + 1) * 128],
                                    xbufs[cidx][:, lc * 256 + c * 128:lc * 256 + (c + 1) * 128],
                                    ident_sb[:])
                        xt = xtp_sb.tile([128, 1024], F16, tag="xt")
                        nc.vector.tensor_copy(xt[:], xtp[:])
                        for pl in range(4):
                            slot = half * 4 + pl
                            q0 = pl * 256
                            q1 = pl * 256 + 128
                            zsl = zp[:, slot * 64:(slot + 1) * 64]
                            nc.tensor.matmul(zsl[0:64, :], xt[:, q0:q0 + 64],
                                             xt[:, q0:q0 + 64],
                                             start=True, stop=False,
                                             skip_group_check=True)
                            nc.tensor.matmul(zsl[64:128, :], xt[:, q0 + 64:q0 + 128],
                                             xt[:, q0 + 64:q0 + 128],
                                             start=True, stop=False,
                                             skip_group_check=True)
                            nc.tensor.matmul(zsl[0:64, :], xt[:, q1:q1 + 64],
                                             xt[:, q1:q1 + 64],
                                             start=False, stop=True,
                                             skip_group_check=True)
                            nc.tensor.matmul(zsl[64:128, :], xt[:, q1 + 64:q1 + 128],
                                             xt[:, q1 + 64:q1 + 128],
                                             start=False, stop=True,
                                             skip_group_check=True)
                    nc.vector.tensor_copy(z_sb[:, q8 * 512:(q8 + 1) * 512], zp[:])

                # ---- PE transpose Z -> batch on partitions -----------------
                # z_sb col = pair*64 + j ; per fixed j take [128=(g,i), 128=pair]
                # (stride 64 elems) -> ztp [128=pair, 128=(g*64+i)].
                # z_t col = (g*64+i)*64 + j.
                zview = z_sb[:].rearrange("p (l j) -> p j l", j=64)
                z_t = ztp_sb.tile([128, st_pairs * 64], F16, tag="zt")
                # dest loop order [p, gi, j8]: inner dim = 8 contiguous fp16
                # (16B runs) so DVE writes bursts, not strided singles.
                ztr = z_t[:].rearrange("p (gi j) -> p gi j", j=64)
                out_sb = osbp.tile([128, 2 * TRI], F32, tag="osb")
                osr = out_sb[:].rearrange("p (g t) -> p g t", g=2)
                ztg = z_t[:].rearrange("p (g c) -> p g c", g=2)
                ydst = y[s * 2 * st_pairs:(s + 1) * 2 * st_pairs, :].rearrange(
                    "(p g) t -> p g t", g=2)
                for jb in range(8):
                    ztp = ztps.tile([128, 1024], F16, tag="ztps")
                    for jj in range(8):
                        nc.tensor.transpose(
                            ztp[:, jj * 128:(jj + 1) * 128],
                            zview[:, jb * 8 + jj, :],
                            ident_sb[:])
                    nc.vector.tensor_copy(
                        ztr[:, :, jb * 8:(jb + 1) * 8],
                        ztp[:].rearrange("p (j gi) -> p gi j", j=8))
                    # triangle rows that only need j <= jb*8+7 (ACT/Pool, no
                    # DMA): overlaps the remaining Z transposes.
                    last = (s == nst - 1)
                    for i in range(jb * 8 + 1, min(jb * 8 + 9, 64)):
                        off = i * (i - 1) // 2
                        dst = osr[:, :, off:off + i]
                        src = ztg[:, :, i * 64:i * 64 + i]
                        # mid-run: GpSimd only (single-phase engines, no
                        # boundary head-of-line). Final supertile: its
                        # compaction is the exposed drain tail and ACT/DVE are
                        # idle, so fan out across all three engines.
                        if not last or i % 3 == 0:
                            nc.gpsimd.tensor_copy(dst, src)
                        elif i % 3 == 1:
                            nc.scalar.copy(dst, src)
                        else:
                            nc.vector.tensor_copy(dst, src)
                    # partial stores overlap the compaction tail; issued from
                    # GpSimd (SWDGE) right behind the compaction copies they
                    # depend on, so no engine queue ever stalls on them. The
                    # final supertile stores in four pieces to shrink the
                    # drain tail.
                    splits = ({3: (0, 528), 5: (528, 1176), 6: (1176, 1596),
                               7: (1596, TRI)} if last
                              else {3: (0, 528), 7: (528, TRI)})
                    if jb in splits:
                        o0, o1 = splits[jb]
                        nc.gpsimd.dma_start(ydst[:, :, o0:o1], osr[:, :, o0:o1])
    return nc


_PROGRAM_CACHE = {}


def _get_program():
    if "nc" not in _PROGRAM_CACHE:
        _PROGRAM_CACHE["nc"] = build_program()
    return _PROGRAM_CACHE["nc"]


def kernel(inputs):
    from concourse.bass_utils import run_bass_kernel_spmd

    x = np.asarray(inputs, dtype=np.float32)
    assert x.shape == (B, N, D), x.shape
    nc = _get_program()
    eye = np.eye(128, dtype=np.float16)
    in_maps = [
        {"x": np.ascontiguousarray(x[i * NB:(i + 1) * NB]), "ident": eye}
        for i in range(NCORES)
    ]
    res = run_bass_kernel_spmd(nc, in_maps, list(range(NCORES)))
    out = np.concatenate([res.results[i]["y"] for i in range(NCORES)], axis=0)
    return out.astype(np.float32, copy=False)


# revision 23
# speedup vs baseline: 1.0522x; 1.0100x over previous
"""DLRM pairwise-interaction kernel for Trainium2 (8 NeuronCores).

Computes, for each batch b: Z_b = X_b @ X_b^T (X_b is [64, 256]) and emits the
strict lower triangle row-major -> [B, 2016] fp32.

Strategy (pure data parallel over B, 1024 batches per core):
  - fp32 inputs loaded with large HWDGE DMAs (sync ring), cast to fp16 on
    DVE/ACT (alternating); dot-product accumulation stays fp32 in PSUM so the
    end-to-end relative error is ~3e-4.
  - PE-transposes pairs of batches ([128,128] fp16 tiles, identity matmul) to
    put the contraction dim (d) on partitions; DVE evacuates PSUM->SBUF.
  - Per batch: Z = XT^T @ XT as two K=128 accumulating matmuls; two batches
    share the 128 weight columns (column-tiled via output base partition 0/64),
    eight pairs of Z accumulate into one [128, 512] fp32 PSUM bank; DVE copies
    to SBUF with a cast to fp16: z_sb[128 part=(g,i), cols=(pair,j)].
  - Triangle extraction without small DMAs (the v1 bottleneck: per-row
    SBUF->SBUF DMAs all landed on 2 of 16 SDMA engines, and ~550 dma_start
    triggers cost ~1us each on the issuing engines):
      * 64 more PE transposes per supertile flip Z to batch-on-partitions:
        z_t[128 part=pair, cols=(g*64+i, j)] (fp16, exact).
      * 63 compute-engine copies (ACT/Pool alternating, g folded into a 3-dim
        AP) compact rows j<i into out_sb[128 part=pair, (g, 2016)] f32.
      * One 2 MB store per supertile with 16 KB contiguous runs per partition
        spreads across all 16 SDMA engines.
"""
import sys
import numpy as np

sys.path.insert(0, "/opt/trn_rl_repo")

import concourse.bass as bass
import concourse.mybir as mybir
import concourse.tile as tile
from concourse.vector_clock import ScopedClock

F32 = mybir.dt.float32
F16 = mybir.dt.float16

B, N, D, TRI = 8192, 64, 256, 2016
NCORES = 8
NB = B // NCORES           # batches per core
ST_PAIRS = 128             # pairs per supertile (=256 batches)
CHUNK_PAIRS = 4            # pairs per input DMA (512 KB fp32 read)

# ---------------------------------------------------------------------------
# Workaround for walrus builds that only accept ONE sync-wait per instruction:
# hoist all-but-one wait onto NoOp instructions committed just before, on the
# same engine (same-engine program order preserves semantics).
# ---------------------------------------------------------------------------
_orig_commit = tile.TileContext._commit_instruction


def _split_waits(self, inst):
    si = getattr(inst, "sync_info", None)
    if si is None or not si.on_wait or len(si.on_wait) <= 1:
        return
    if inst.engine == mybir.EngineType.Unassigned:
        return
    waits = list(si.on_wait)
    inst.sync_info = mybir.SyncInfo(on_wait=[waits[-1]], on_update=list(si.on_update))
    for w in waits[:-1]:
        nop = mybir.InstNoOp(name=f"{inst.name}-wsplit-{w.id}", ins=[], outs=[])
        nop.engine = inst.engine
        nop.sync_info = mybir.SyncInfo(on_wait=[w], on_update=[])
        _orig_commit(self, nop, lazy_reg_writes=False)


def _commit_instruction_split(self, inst, lazy_reg_writes=True):
    _split_waits(self, inst)
    return _orig_commit(self, inst, lazy_reg_writes=lazy_reg_writes)


def _drain_and_barrier_split(self, tick_clock, wait_clock):
    drain_inst = self.nc.sync.drain()
    wait_clock.add_sem_waits(
        drain_inst.ins, ScopedClock({None: tick_clock.global_clock})
    )
    si = drain_inst.ins.sync_info
    if si is not None and si.on_wait and len(si.on_wait) > 1:
        waits = list(si.on_wait)
        drain_inst.ins.sync_info = mybir.SyncInfo(
            on_wait=[waits[0]], on_update=list(si.on_update)
        )
        for w in waits[1:]:
            nop = self.nc.sync.nop(nofuse=True)
            nop.ins.sync_info = mybir.SyncInfo(on_wait=[w], on_update=[])

    self.nc.all_engine_barrier()
    assert self.sems is not None
    popped = self.nc._tile_sem_poison_stack.pop()
    assert popped is self._sem_poison
    self.nc.clear_and_free_semaphores(list(self.sems.allocated().values()))
    self.nc.all_engine_barrier()


def _install_tile_workarounds():
    tile.TileContext._commit_instruction = _commit_instruction_split
    tile.TileContext._drain_and_barrier = _drain_and_barrier_split


def build_program(nb=NB, st_pairs=ST_PAIRS, chunk_pairs=CHUNK_PAIRS):
    _install_tile_workarounds()
    npairs = nb // 2
    nst = npairs // st_pairs
    nchunks = st_pairs // chunk_pairs
    assert st_pairs == 128

    nc = bass.Bass("TRN2", target_bir_lowering=False, debug=False,
                   num_devices=NCORES)
    x = nc.dram_tensor("x", [nb, N, D], F32, kind="ExternalInput").ap()
    ident = nc.dram_tensor("ident", [128, 128], F16, kind="ExternalInput").ap()
    y = nc.dram_tensor("y", [nb, TRI], F32, kind="ExternalOutput").ap()
    xflat = x.rearrange("b n d -> (b n) d")

    with tile.TileContext(nc) as tc:
        with (
            tc.tile_pool(name="const", bufs=1) as constp,
            tc.tile_pool(name="xf32", bufs=8) as xf32p,
            tc.tile_pool(name="xin", bufs=10) as xinp,
            tc.tile_pool(name="xt", bufs=3) as xtp_sb,
            tc.tile_pool(name="zsb", bufs=2) as zsbp,
            tc.tile_pool(name="zt", bufs=2) as ztp_sb,
            tc.tile_pool(name="osb", bufs=2) as osbp,
            tc.tile_pool(name="xtps", bufs=2, space="PSUM") as xtps,
            tc.tile_pool(name="zps", bufs=3, space="PSUM") as zps,
            tc.tile_pool(name="ztps", bufs=3, space="PSUM") as ztps,
        ):
            ident_sb = constp.tile([128, 128], F16)
            nc.sync.dma_start(ident_sb[:], ident[:])

            for s in range(nst):
                # ---- load X (fp32, sync HWDGE ring) + cast on DVE/ACT ----
                xbufs = []
                for c in range(nchunks):
                    xf = xf32p.tile([128, chunk_pairs * 256], F32, tag="xf32")
                    row0 = (s * st_pairs + c * chunk_pairs) * 128
                    src = xflat[row0:row0 + chunk_pairs * 128, :].rearrange(
                        "(l p) d -> p l d", p=128)
                    nc.sync.dma_start(
                        xf[:].rearrange("p (l d) -> p l d", d=256), src)
                    xb = xinp.tile([128, chunk_pairs * 256], F16, tag="xin")
                    # all casts on ACT: keeps every engine single-phase so a
                    # supertile boundary never head-of-line blocks the next
                    # supertile's early work.
                    nc.scalar.copy(xb[:], xf[:])
                    xbufs.append(xb)

                # ---- Z = X X^T per pair; z_sb fp16 [(g,i), (pair,j)] -------
                z_sb = zsbp.tile([128, st_pairs * 64], F16, tag="zsb")
                for q8 in range(st_pairs // 8):
                    zp = zps.tile([128, 512], F32, tag="zps")
                    for half in range(2):
                        q4 = q8 * 2 + half
                        xtp = xtps.tile([128, 1024], F16, tag="xtps")
                        for pl in range(4):
                            l = q4 * 4 + pl
                            cidx, lc = divmod(l, chunk_pairs)
                            for c in range(2):
                                nc.tensor.transpose(
                                    xtp[:, pl * 256 + c * 128:pl * 256 + (c + 1) * 128],
                                    xbufs[cidx][:, lc * 256 + c * 128:lc * 256 + (c + 1) * 128],
                                    ident_sb[:])
                        xt = xtp_sb.tile([128, 1024], F16, tag="xt")
                        nc.vector.tensor_copy(xt[:], xtp[:])
                        for pl in range(4):
                            slot = half * 4 + pl
                            q0 = pl * 256
                            q1 = pl * 256 + 128
                            zsl = zp[:, slot * 64:(slot + 1) * 64]
                            nc.tensor.matmul(zsl[0:64, :], xt[:, q0:q0 + 64],
                                             xt[:, q0:q0 + 64],
                                             start=True, stop=False,
                                             skip_group_check=True)
                            nc.tensor.matmul(zsl[64:128, :], xt[:, q0 + 64:q0 + 128],
                                             xt[:, q0 + 64:q0 + 128],
                                             start=True, stop=False,
                                             skip_group_check=True)
                            nc.tensor.matmul(zsl[0:64, :], xt[:, q1:q1 + 64],
                                             xt[:, q1:q1 + 64],
                                             start=False, stop=True,
                                             skip_group_check=True)
                            nc.tensor.matmul(zsl[64:128, :], xt[:, q1 + 64:q1 + 128],
                                             xt[:, q1 + 64:q1 + 128],
                                             start=False, stop=True,
                                             skip_group_check=True)
                    nc.vector.tensor_copy(z_sb[:, q8 * 512:(q8 + 1) * 512], zp[:])

                # ---- PE transpose Z -> batch on partitions -----------------
                # z_sb col = pair*64 + j ; per fixed j take [128=(g,i), 128=pair]
                # (stride 64 elems) -> ztp [128=pair, 128=(g*64+i)].
                # z_t col = (g*64+i)*64 + j.
                zview = z_sb[:].rearrange("p (l j) -> p j l", j=64)
                z_t = ztp_sb.tile([128, st_pairs * 64], F16, tag="zt")
                # dest loop order [p, gi, j8]: inner dim = 8 contiguous fp16
                # (16B runs) so DVE writes bursts, not strided singles.
                ztr = z_t[:].rearrange("p (gi j) -> p gi j", j=64)
                out_sb = osbp.tile([128, 2 * TRI], F32, tag="osb")
                osr = out_sb[:].rearrange("p (g t) -> p g t", g=2)
                ztg = z_t[:].rearrange("p (g c) -> p g c", g=2)
                ydst = y[s * 2 * st_pairs:(s + 1) * 2 * st_pairs, :].rearrange(
                    "(p g) t -> p g t", g=2)
                for jb in range(8):
                    ztp = ztps.tile([128, 1024], F16, tag="ztps")
                    for jj in range(8):
                        nc.tensor.transpose(
                            ztp[:, jj * 128:(jj + 1) * 128],
                            zview[:, jb * 8 + jj, :],
                            ident_sb[:])
                    nc.vector.tensor_copy(
                        ztr[:, :, jb * 8:(jb + 1) * 8],
                        ztp[:].rearrange("p (j gi) -> p gi j", j=8))
                    # triangle rows that only need j <= jb*8+7 (ACT/Pool, no
                    # DMA): overlaps the remaining Z transposes.
                    last = (s == nst - 1)
                    for i in range(jb * 8 + 1, min(jb * 8 + 9, 64)):
                        off = i * (i - 1) // 2
                        dst = osr[:, :, off:off + i]
                        src = ztg[:, :, i * 64:i * 64 + i]
                        # mid-run: GpSimd only (single-phase engines, no
                        # boundary head-of-line). Final supertile: its
                        # compaction is the exposed drain tail and ACT/DVE are
                        # idle, so fan out across all three engines.
                        if not last or i % 3 == 0:
                            nc.gpsimd.tensor_copy(dst, src)
                        elif i % 3 == 1:
                            nc.scalar.copy(dst, src)
                        else:
                            nc.vector.tensor_copy(dst, src)
                    # partial stores overlap the compaction tail; issued from
                    # GpSimd (SWDGE) right behind the compaction copies they
                    # depend on, so no engine queue ever stalls on them. The
                    # final supertile stores in four pieces to shrink the
                    # drain tail.
                    splits = ({3: (0, 528), 5: (528, 1176), 6: (1176, 1596),
                               7: (1596, TRI)} if last
                              else {3: (0, 528), 7: (528, TRI)})
                    if jb in splits:
                        o0, o1 = splits[jb]
                        nc.gpsimd.dma_start(ydst[:, :, o0:o1], osr[:, :, o0:o1])
    return nc


_PROGRAM_CACHE = {}


def _get_program():
    if "nc" not in _PROGRAM_CACHE:
        _PROGRAM_CACHE["nc"] = build_program()
    return _PROGRAM_CACHE["nc"]


def kernel(inputs):
    from concourse.bass_utils import run_bass_kernel_spmd

    x = np.asarray(inputs, dtype=np.float32)
    assert x.shape == (B, N, D), x.shape
    nc = _get_program()
    eye = np.eye(128, dtype=np.float16)
    in_maps = [
        {"x": np.ascontiguousarray(x[i * NB:(i + 1) * NB]), "ident": eye}
        for i in range(NCORES)
    ]
    res = run_bass_kernel_spmd(nc, in_maps, list(range(NCORES)))
    out = np.concatenate([res.results[i]["y"] for i in range(NCORES)], axis=0)
    return out.astype(np.float32, copy=False)
